# revision 11
# baseline (speedup 1.0000x reference)
"""Trainium2 Bass kernel for CustomConv: 3x3 conv (pad=1, stride=1) + bias + ReLU.

Input  prev_a  [32, 56, 56, 128] f32 (NHWC)
       filter_w [3, 3, 128, 256] f32 (HWIO)
       filter_b [1, 1, 1, 256]   f32
Output [32, 56, 56, 256] f32

Strategy: data-parallel over batch (4 images per core on 8 cores).
Host pre-transposes to NCHW with a 1-px zero-padded ring so each of the
9 filter taps is a strided SBUF view; conv = 9 accumulated matmuls per
output tile (contraction over the 128 input channels on the partition
dim). Matmuls run in fp16 (10 mantissa bits, fp32 PSUM accumulation).

Weight-stationary: the tap loop is outside the row-group loop so one
LDWEIGHTS covers a group of matmuls and the PE streams at the 448-
cycle floor. The head is latency-tuned: input-x DMAs ride the sync
queue group and weights the scalar group, so the two trigger streams
issue in parallel and each group's per-ring FIFO gives the critical
first bytes (weight taps 0-2, input rows 0-10) priority over the rest.
Zero-tile warmup matmuls keep the PE busy from the body start until
data lands, pre-warming the HAM clock gate with no gap. Output blocks
alternate the gpsimd/scalar rings; the final row group is split into
two half-column activations + sync DMAs so the drain is short. Output
is stored fp16 (halves write traffic; the host upcasts). Note: the
runtime's end-of-kernel semaphore-file reset (~51 serialized clears
per engine, ~6us on the tensor engine) is a fixed tax independent of
program structure.
"""
import numpy as np

import concourse.tile as tile
from concourse import bacc, mybir
from concourse import bass_utils

# Disable walrus birsim (compile-time simulation of the kernel). The
# NEFF produced is identical; this only skips a slow verification step.
_orig_run_command = bass_utils.run_command


def _no_birsim_run_command(argv, **kwargs):
    argv = ["--enable-birsim=false" if a == "--enable-birsim=true" else a
            for a in argv]
    return _orig_run_command(argv, **kwargs)


bass_utils.run_command = _no_birsim_run_command

N_CORES = 8
IMG_PER_CORE = 4
H = 56          # output spatial
HP = 58         # padded input spatial
CIN = 128
COUT = 256
TAPS = [(dy, dx) for dy in range(3) for dx in range(3)]
RG_ROWS = 8     # output rows per group
NFREE = RG_ROWS * H  # 448 positions per matmul (<= 512 PSUM bank)
# (first row group, group size, j) blocks per image. Image 0 starts
# with single-row-group blocks so matmuls begin as soon as a 10-row
# input prefix and the first weight taps land; the last image tapers
# (4,2,1) so the drain after the last matmul is one small act + DMA.
BLOCKS = {
    0: [(0, 1, 0), (1, 1, 0), (2, 1, 0), (3, 1, 0), (4, 3, 0),
        (0, 2, 1), (2, 2, 1), (4, 3, 1)],
    1: [(0, 4, 0), (0, 4, 1), (4, 3, 0), (4, 3, 1)],
    2: [(0, 4, 0), (0, 4, 1), (4, 3, 0), (4, 3, 1)],
    3: [(0, 4, 0), (0, 4, 1), (4, 2, 0), (4, 2, 1), (6, 1, 0), (6, 1, 1)],
}
# image-0 row chunks: [0,10) feeds row-group 0; each later chunk
# unlocks the next row-group(s) while earlier groups compute.
X0_CHUNKS = [(0, 10), (10, 30), (30, 58)]
N_WARM = 18     # N=128 warmup matmuls (~107ns each cold)

TRACE = False
TRACE_KWARGS = {}
LAST_RESULTS = None
_NC_CACHE = None


def _build():
    nc = bacc.Bacc("TRN2", debug=False, target_bir_lowering=False,
                   num_devices=N_CORES, enable_partition_id=False,
                   monotonic_sem_count=0)
    x_d = nc.dram_tensor("x", [IMG_PER_CORE, CIN, HP, HP],
                         mybir.dt.float16, kind="ExternalInput")
    w_d = nc.dram_tensor("w", [CIN, 2, 9 * 128 + 1],
                         mybir.dt.float16, kind="ExternalInput")
    o_d = nc.dram_tensor("o", [IMG_PER_CORE, 2, 128, H * H],
                         mybir.dt.float16, kind="ExternalOutput")

    with tile.TileContext(nc) as tc:
        with (tc.tile_pool(name="wb", bufs=10) as wbp,
              tc.tile_pool(name="x", bufs=4) as xp,
              tc.tile_pool(name="o", bufs=8) as op,
              tc.tile_pool(name="ps", bufs=8, space="PSUM") as pp):
            wt = wbp.tile([CIN, 2, 9 * 128 + 1], mybir.dt.float16, tag="wtap")
            xts = [xp.tile([CIN, HP, HP], mybir.dt.float16,
                           tag="ximg", name=f"ximg{k}") for k in range(2)]
            gt = wbp.tile([CIN, 128], mybir.dt.float16, tag="warm")

            # Head DMAs, critical-first. x on the sync group, w on the
            # scalar group: triggers issue in parallel on the two
            # engines and each group's per-ring FIFO lands the first
            # chunk of each stream before the rest. A tiny dummy DMA
            # leads each group so the ~0.8us DGE pipeline-fill latency
            # is absorbed before the critical chunk's packets.
            nc.sync.dma_start(xts[0][:, 0:1, 0:1], x_d.ap()[0, :, 0:1, 0:1])
            for (a, b) in X0_CHUNKS:
                nc.sync.dma_start(xts[0][:, a:b, :], x_d.ap()[0, :, a:b, :])
            nc.sync.dma_start(xts[1][:], x_d.ap()[1])  # image-1 prefetch
            nc.scalar.dma_start(wt[:, 0, 0:1], w_d.ap()[:, 0, 0:1])
            nc.scalar.dma_start(wt[:, 0, 0:384], w_d.ap()[:, 0, 0:384])
            nc.scalar.dma_start(wt[:, 0, 384:], w_d.ap()[:, 0, 384:])
            nc.scalar.dma_start(wt[:, 1], w_d.ap()[:, 1])

            # 8 PSUM accumulators (one bank each) rotating across blocks
            pss = [pp.tile([128, NFREE], mybir.dt.float32,
                           tag="psg", name=f"psg{k}") for k in range(8)]
            # output staging, one per (group, j) block in flight
            ots = [op.tile([128, 4 * NFREE], mybir.dt.float16,
                           tag="og", name=f"og{k}") for k in range(4)]

            # pre-warm the PE clock gate (HAM) with small matmuls on a
            # dedicated scratch tile while the first input DMAs are in
            # flight, so real matmuls run at 2.4 GHz almost from the
            # start. They write psum bank 0; the first real tap uses
            # start=True so the garbage never reaches an accumulation.
            # The memset rides gpsimd (the engine released earliest
            # from the preamble barrier) so warmups start ~6.5us with
            # no gap between the last warmup and the first real matmul
            # (a gap resets the HAM busy window).
            nc.gpsimd.memset(gt[:], 0.0)
            for _ in range(N_WARM):
                nc.tensor.matmul(pss[0][:, 0:128], gt[:], gt[:],
                                 start=True, stop=True)

            bank = 0
            blk = 0
            for img in range(IMG_PER_CORE):
                xt = xts[img % 2]
                for (g0, gsz, j) in BLOCKS[img]:
                    ot = ots[blk % 4]
                    bias_ap = wt[:, j, 9 * 128:9 * 128 + 1]
                    is_last = img == 3 and (g0, j) == (6, 1)
                    if is_last:
                        # final row group as two 4-row (N=224) PSUM
                        # sub-blocks: sub-A's whole act+DMA drain hides
                        # under sub-B's ~1.7us of matmuls (same total
                        # matmul cycles), leaving only the small sub-B
                        # flush exposed at the end.
                        hf = NFREE // 2
                        for h in range(2):
                            bnk = pss[(bank + h) % 8]
                            for t, (dy, dx) in enumerate(TAPS):
                                w_ap = wt[:, j, t * 128:(t + 1) * 128]
                                r0 = g0 * RG_ROWS + h * 4 + dy
                                nc.tensor.matmul(
                                    bnk[:, 0:hf], w_ap,
                                    xt[:, r0:r0 + 4, dx:dx + H],
                                    start=(t == 0), stop=(t == 8),
                                )
                            nc.scalar.activation(
                                ot[:, h * hf:(h + 1) * hf], bnk[:, 0:hf],
                                mybir.ActivationFunctionType.Relu,
                                bias=bias_ap)
                            nc.sync.dma_start(
                                o_d.ap()[img, j, :,
                                         g0 * NFREE + h * hf:
                                         g0 * NFREE + (h + 1) * hf],
                                ot[:, h * hf:(h + 1) * hf])
                        bank += 2
                        blk += 1
                        continue
                    banks = [pss[(bank + k) % 8] for k in range(gsz)]
                    bank += gsz
                    for t, (dy, dx) in enumerate(TAPS):
                        w_ap = wt[:, j, t * 128:(t + 1) * 128]
                        for k in range(gsz):
                            r0 = (g0 + k) * RG_ROWS + dy
                            nc.tensor.matmul(
                                banks[k][:], w_ap,
                                xt[:, r0:r0 + RG_ROWS, dx:dx + H],
                                start=(t == 0), stop=(t == 8),
                            )
                    for k in range(gsz):
                        nc.scalar.activation(
                            ot[:, k * NFREE:(k + 1) * NFREE], banks[k][:],
                            mybir.ActivationFunctionType.Relu,
                            bias=bias_ap)
                    # alternate output DMAs across the gpsimd and
                    # scalar rings; sync stays clear for input
                    # prefetches and the final drain
                    oeng = nc.gpsimd if blk % 2 == 0 else nc.scalar
                    blk += 1
                    oeng.dma_start(
                        o_d.ap()[img, j, :,
                                 g0 * NFREE:(g0 + gsz) * NFREE],
                        ot[:, 0:gsz * NFREE])
                # Prefetch image img+2 only after this image's blocks:
                # its WAR wait (previous reads of the same buffer) is
                # then already satisfied when it reaches the FIFO head,
                # so it never blocks the final drain DMAs in the sync
                # FIFO.
                if img < 2:
                    nc.sync.dma_start(xts[img % 2][:], x_d.ap()[img + 2])
    nc.compile()
    return nc


def kernel(prev_a, filter_w, filter_b):
    global LAST_RESULTS, _NC_CACHE
    from concourse.bass_utils import run_bass_kernel_spmd

    prev_a = np.asarray(prev_a, dtype=np.float32)
    filter_w = np.asarray(filter_w, dtype=np.float32)
    filter_b = np.asarray(filter_b, dtype=np.float32)

    n = prev_a.shape[0]
    xpad = np.zeros((n, CIN, HP, HP), dtype=np.float16)
    xpad[:, :, 1:1 + H, 1:1 + H] = prev_a.transpose(0, 3, 1, 2).astype(np.float16)
    # [cin, j-half, tap*128 + bias] so each cout half (taps + its bias
    # column) is one contiguous region
    w = np.zeros((CIN, 2, 9 * 128 + 1), dtype=np.float16)
    w[:, :, :9 * 128] = (filter_w.transpose(2, 0, 1, 3)
                         .reshape(CIN, 9, 2, 128).transpose(0, 2, 1, 3)
                         .reshape(CIN, 2, 9 * 128).astype(np.float16))
    w[:, :, 9 * 128] = filter_b.reshape(2, 128).T.astype(np.float16)

    if _NC_CACHE is None:
        _NC_CACHE = _build()
    nc = _NC_CACHE

    in_maps = [
        {"x": np.ascontiguousarray(xpad[c * IMG_PER_CORE:(c + 1) * IMG_PER_CORE]),
         "w": w}
        for c in range(N_CORES)
    ]
    LAST_RESULTS = run_bass_kernel_spmd(
        nc, in_maps, core_ids=list(range(N_CORES)), trace=TRACE,
        **TRACE_KWARGS)

    outs = []
    for c in range(N_CORES):
        o = LAST_RESULTS.results[c]["o"]  # [4, 2, 128, 3136] fp16
        outs.append(o.astype(np.float32)
                    .reshape(IMG_PER_CORE, COUT, H, H).transpose(0, 2, 3, 1))
    return np.ascontiguousarray(np.concatenate(outs, axis=0))


# revision 13
# speedup vs baseline: 1.0156x; 1.0156x over previous
"""Trainium2 Bass kernel for CustomConv: 3x3 conv (pad=1, stride=1) + bias + ReLU.

Input  prev_a  [32, 56, 56, 128] f32 (NHWC)
       filter_w [3, 3, 128, 256] f32 (HWIO)
       filter_b [1, 1, 1, 256]   f32
Output [32, 56, 56, 256] f32

Strategy: data-parallel over batch (4 images per core on 8 cores).
Host pre-transposes to NCHW with a 1-px zero-padded ring so each of the
9 filter taps is a strided SBUF view; conv = 9 accumulated matmuls per
output tile (contraction over the 128 input channels on the partition
dim). Matmuls run in fp16 (10 mantissa bits, fp32 PSUM accumulation).

Weight-stationary: the tap loop is outside the row-group loop so one
LDWEIGHTS covers a group of matmuls and the PE streams at the 448-
cycle floor. The head is latency-tuned: input-x DMAs ride the sync
queue group and weights the scalar group, so the two trigger streams
issue in parallel and each group's per-ring FIFO gives the critical
first bytes (weight taps 0-2, input rows 0-10) priority over the rest.
Zero-tile warmup matmuls keep the PE busy from the body start until
data lands, pre-warming the HAM clock gate with no gap. Output blocks
alternate the gpsimd/scalar rings; the final row group is split into
two half-column activations + sync DMAs so the drain is short. Output
is stored fp16 (halves write traffic; the host upcasts). Note: the
runtime's end-of-kernel semaphore-file reset (~51 serialized clears
per engine, ~6us on the tensor engine) is a fixed tax independent of
program structure.
"""
import numpy as np

import concourse.tile as tile
from concourse import bacc, mybir
from concourse import bass_utils

# Disable walrus birsim (compile-time simulation of the kernel). The
# NEFF produced is identical; this only skips a slow verification step.
_orig_run_command = bass_utils.run_command


def _no_birsim_run_command(argv, **kwargs):
    argv = ["--enable-birsim=false" if a == "--enable-birsim=true" else a
            for a in argv]
    return _orig_run_command(argv, **kwargs)


bass_utils.run_command = _no_birsim_run_command

N_CORES = 8
IMG_PER_CORE = 4
H = 56          # output spatial
HP = 58         # padded input spatial
CIN = 128
COUT = 256
TAPS = [(dy, dx) for dy in range(3) for dx in range(3)]
RG_ROWS = 8     # output rows per group
NFREE = RG_ROWS * H  # 448 positions per matmul (<= 512 PSUM bank)
# (first row group, group size, j) blocks per image. Image 0 starts
# with single-row-group blocks so matmuls begin as soon as a 10-row
# input prefix and the first weight taps land; the last image tapers
# (4,2,1) so the drain after the last matmul is one small act + DMA.
BLOCKS = {
    0: [(0, 1, 0), (1, 1, 0), (2, 1, 0), (3, 1, 0), (4, 3, 0),
        (0, 2, 1), (2, 2, 1), (4, 3, 1)],
    1: [(0, 4, 0), (0, 4, 1), (4, 3, 0), (4, 3, 1)],
    2: [(0, 4, 0), (0, 4, 1), (4, 3, 0), (4, 3, 1)],
    3: [(0, 4, 0), (0, 4, 1), (4, 2, 0), (4, 2, 1), (6, 1, 0), (6, 1, 1)],
}
# image-0 row chunks: [0,10) feeds row-group 0; each later chunk
# unlocks the next row-group(s) while earlier groups compute.
X0_CHUNKS = [(0, 10), (10, 30), (30, 58)]
N_WARM = 20     # N=128 warmup matmuls (~107ns each cold)

TRACE = False
TRACE_KWARGS = {}
LAST_RESULTS = None
_NC_CACHE = None


def _build():
    nc = bacc.Bacc("TRN2", debug=False, target_bir_lowering=False,
                   num_devices=N_CORES, enable_partition_id=False,
                   monotonic_sem_count=0)
    x_d = nc.dram_tensor("x", [IMG_PER_CORE, CIN, HP, HP],
                         mybir.dt.float16, kind="ExternalInput")
    w_d = nc.dram_tensor("w", [CIN, 2, 9 * 128 + 1],
                         mybir.dt.float16, kind="ExternalInput")
    o_d = nc.dram_tensor("o", [IMG_PER_CORE, 2, 128, H * H],
                         mybir.dt.float16, kind="ExternalOutput")

    with tile.TileContext(nc) as tc:
        with (tc.tile_pool(name="wb", bufs=10) as wbp,
              tc.tile_pool(name="x", bufs=4) as xp,
              tc.tile_pool(name="o", bufs=8) as op,
              tc.tile_pool(name="ps", bufs=8, space="PSUM") as pp):
            wt = wbp.tile([CIN, 2, 9 * 128 + 1], mybir.dt.float16, tag="wtap")
            xts = [xp.tile([CIN, HP, HP], mybir.dt.float16,
                           tag="ximg", name=f"ximg{k}") for k in range(2)]
            gt = wbp.tile([CIN, 128], mybir.dt.float16, tag="warm")
            dmy = wbp.tile([CIN, 2], mybir.dt.float16, tag="dmy")

            # Head DMAs, critical-first. x on the sync group, w on the
            # scalar group: triggers issue in parallel on the two
            # engines and each group's per-ring FIFO lands the first
            # chunk of each stream before the rest. A tiny dummy DMA
            # into a dedicated scratch tile leads each group so the
            # ~0.8us DGE pipeline-fill latency is absorbed before the
            # critical chunk's packets (the scratch target must be
            # disjoint from everything else or the Tile scheduler
            # reorders the real chunks behind the dummy's completion).
            nc.sync.dma_start(dmy[:, 0:1], x_d.ap()[0, :, 0:1, 0:1])
            for (a, b) in X0_CHUNKS:
                nc.sync.dma_start(xts[0][:, a:b, :], x_d.ap()[0, :, a:b, :])
            nc.sync.dma_start(xts[1][:], x_d.ap()[1])  # image-1 prefetch
            nc.scalar.dma_start(dmy[:, 1:2], w_d.ap()[:, 0, 0:1])
            nc.scalar.dma_start(wt[:, 0, 0:384], w_d.ap()[:, 0, 0:384])
            nc.scalar.dma_start(wt[:, 0, 384:], w_d.ap()[:, 0, 384:])
            nc.scalar.dma_start(wt[:, 1], w_d.ap()[:, 1])

            # 8 PSUM accumulators (one bank each) rotating across blocks
            pss = [pp.tile([128, NFREE], mybir.dt.float32,
                           tag="psg", name=f"psg{k}") for k in range(8)]
            # output staging, one per (group, j) block in flight
            ots = [op.tile([128, 4 * NFREE], mybir.dt.float16,
                           tag="og", name=f"og{k}") for k in range(4)]

            # pre-warm the PE clock gate (HAM) with small matmuls on a
            # dedicated scratch tile while the first input DMAs are in
            # flight, so real matmuls run at 2.4 GHz almost from the
            # start. They write psum bank 0; the first real tap uses
            # start=True so the garbage never reaches an accumulation.
            # The memset rides gpsimd (the engine released earliest
            # from the preamble barrier) so warmups start ~6.5us with
            # no gap between the last warmup and the first real matmul
            # (a gap resets the HAM busy window).
            nc.gpsimd.memset(gt[:], 0.0)
            for _ in range(N_WARM):
                nc.tensor.matmul(pss[0][:, 0:128], gt[:], gt[:],
                                 start=True, stop=True)

            bank = 0
            blk = 0
            for img in range(IMG_PER_CORE):
                xt = xts[img % 2]
                for (g0, gsz, j) in BLOCKS[img]:
                    ot = ots[blk % 4]
                    bias_ap = wt[:, j, 9 * 128:9 * 128 + 1]
                    is_last = img == 3 and (g0, j) == (6, 1)
                    if is_last:
                        # final row group as two 4-row (N=224) PSUM
                        # sub-blocks: sub-A's whole act+DMA drain hides
                        # under sub-B's ~1.7us of matmuls (same total
                        # matmul cycles), leaving only the small sub-B
                        # flush exposed at the end.
                        hf = NFREE // 2
                        for h in range(2):
                            bnk = pss[(bank + h) % 8]
                            for t, (dy, dx) in enumerate(TAPS):
                                w_ap = wt[:, j, t * 128:(t + 1) * 128]
                                r0 = g0 * RG_ROWS + h * 4 + dy
                                nc.tensor.matmul(
                                    bnk[:, 0:hf], w_ap,
                                    xt[:, r0:r0 + 4, dx:dx + H],
                                    start=(t == 0), stop=(t == 8),
                                )
                            nc.scalar.activation(
                                ot[:, h * hf:(h + 1) * hf], bnk[:, 0:hf],
                                mybir.ActivationFunctionType.Relu,
                                bias=bias_ap)
                            nc.sync.dma_start(
                                o_d.ap()[img, j, :,
                                         g0 * NFREE + h * hf:
                                         g0 * NFREE + (h + 1) * hf],
                                ot[:, h * hf:(h + 1) * hf])
                        bank += 2
                        blk += 1
                        continue
                    banks = [pss[(bank + k) % 8] for k in range(gsz)]
                    bank += gsz
                    for t, (dy, dx) in enumerate(TAPS):
                        w_ap = wt[:, j, t * 128:(t + 1) * 128]
                        for k in range(gsz):
                            r0 = (g0 + k) * RG_ROWS + dy
                            nc.tensor.matmul(
                                banks[k][:], w_ap,
                                xt[:, r0:r0 + RG_ROWS, dx:dx + H],
                                start=(t == 0), stop=(t == 8),
                            )
                    for k in range(gsz):
                        nc.scalar.activation(
                            ot[:, k * NFREE:(k + 1) * NFREE], banks[k][:],
                            mybir.ActivationFunctionType.Relu,
                            bias=bias_ap)
                    # alternate output DMAs across the gpsimd and
                    # scalar rings; sync stays clear for input
                    # prefetches and the final drain
                    oeng = nc.gpsimd if blk % 2 == 0 else nc.scalar
                    blk += 1
                    oeng.dma_start(
                        o_d.ap()[img, j, :,
                                 g0 * NFREE:(g0 + gsz) * NFREE],
                        ot[:, 0:gsz * NFREE])
                # Prefetch image img+2 only after this image's blocks:
                # its WAR wait (previous reads of the same buffer) is
                # then already satisfied when it reaches the FIFO head,
                # so it never blocks the final drain DMAs in the sync
                # FIFO.
                if img < 2:
                    nc.sync.dma_start(xts[img % 2][:], x_d.ap()[img + 2])
    nc.compile()
    return nc


def kernel(prev_a, filter_w, filter_b):
    global LAST_RESULTS, _NC_CACHE
    from concourse.bass_utils import run_bass_kernel_spmd

    prev_a = np.asarray(prev_a, dtype=np.float32)
    filter_w = np.asarray(filter_w, dtype=np.float32)
    filter_b = np.asarray(filter_b, dtype=np.float32)

    n = prev_a.shape[0]
    xpad = np.zeros((n, CIN, HP, HP), dtype=np.float16)
    xpad[:, :, 1:1 + H, 1:1 + H] = prev_a.transpose(0, 3, 1, 2).astype(np.float16)
    # [cin, j-half, tap*128 + bias] so each cout half (taps + its bias
    # column) is one contiguous region
    w = np.zeros((CIN, 2, 9 * 128 + 1), dtype=np.float16)
    w[:, :, :9 * 128] = (filter_w.transpose(2, 0, 1, 3)
                         .reshape(CIN, 9, 2, 128).transpose(0, 2, 1, 3)
                         .reshape(CIN, 2, 9 * 128).astype(np.float16))
    w[:, :, 9 * 128] = filter_b.reshape(2, 128).T.astype(np.float16)

    if _NC_CACHE is None:
        _NC_CACHE = _build()
    nc = _NC_CACHE

    in_maps = [
        {"x": np.ascontiguousarray(xpad[c * IMG_PER_CORE:(c + 1) * IMG_PER_CORE]),
         "w": w}
        for c in range(N_CORES)
    ]
    LAST_RESULTS = run_bass_kernel_spmd(
        nc, in_maps, core_ids=list(range(N_CORES)), trace=TRACE,
        **TRACE_KWARGS)

    outs = []
    for c in range(N_CORES):
        o = LAST_RESULTS.results[c]["o"]  # [4, 2, 128, 3136] fp16
        outs.append(o.astype(np.float32)
                    .reshape(IMG_PER_CORE, COUT, H, H).transpose(0, 2, 3, 1))
    return np.ascontiguousarray(np.concatenate(outs, axis=0))


# revision 15
# speedup vs baseline: 1.0268x; 1.0110x over previous
"""Trainium2 Bass kernel for CustomConv: 3x3 conv (pad=1, stride=1) + bias + ReLU.

Input  prev_a  [32, 56, 56, 128] f32 (NHWC)
       filter_w [3, 3, 128, 256] f32 (HWIO)
       filter_b [1, 1, 1, 256]   f32
Output [32, 56, 56, 256] f32

Strategy: data-parallel over batch (4 images per core on 8 cores).
Host pre-transposes to NCHW with a 1-px zero-padded ring so each of the
9 filter taps is a strided SBUF view; conv = 9 accumulated matmuls per
output tile (contraction over the 128 input channels on the partition
dim). Matmuls run in fp16 (10 mantissa bits, fp32 PSUM accumulation).

Weight-stationary: the tap loop is outside the row-group loop so one
LDWEIGHTS covers a group of matmuls and the PE streams at the 448-
cycle floor. The head is latency-tuned: input-x DMAs ride the sync
queue group and weights the scalar group, so the two trigger streams
issue in parallel and each group's per-ring FIFO gives the critical
first bytes (weight taps 0-2, input rows 0-10) priority over the rest.
Zero-tile warmup matmuls keep the PE busy from the body start until
data lands, pre-warming the HAM clock gate with no gap. Output blocks
alternate the gpsimd/scalar rings; the final row group is split into
two half-column activations + sync DMAs so the drain is short. Output
is stored fp16 (halves write traffic; the host upcasts). Note: the
runtime's end-of-kernel semaphore-file reset (~51 serialized clears
per engine, ~6us on the tensor engine) is a fixed tax independent of
program structure.
"""
import numpy as np

import concourse.tile as tile
from concourse import bacc, mybir
from concourse import bass_utils

# Disable walrus birsim (compile-time simulation of the kernel). The
# NEFF produced is identical; this only skips a slow verification step.
_orig_run_command = bass_utils.run_command


def _no_birsim_run_command(argv, **kwargs):
    argv = ["--enable-birsim=false" if a == "--enable-birsim=true" else a
            for a in argv]
    return _orig_run_command(argv, **kwargs)


bass_utils.run_command = _no_birsim_run_command

N_CORES = 8
IMG_PER_CORE = 4
H = 56          # output spatial
HP = 58         # padded input spatial
CIN = 128
COUT = 256
TAPS = [(dy, dx) for dy in range(3) for dx in range(3)]
RG_ROWS = 8     # output rows per group
NFREE = RG_ROWS * H  # 448 positions per matmul (<= 512 PSUM bank)
# (first row group, group size, j) blocks per image. Image 0 starts
# with single-row-group blocks so matmuls begin as soon as a 10-row
# input prefix and the first weight taps land; the last image tapers
# (4,2,1) so the drain after the last matmul is one small act + DMA.
BLOCKS = {
    0: [(0, 1, 0), (1, 1, 0), (2, 1, 0), (3, 1, 0), (4, 3, 0),
        (0, 2, 1), (2, 2, 1), (4, 3, 1)],
    1: [(0, 4, 0), (0, 4, 1), (4, 3, 0), (4, 3, 1)],
    2: [(0, 4, 0), (0, 4, 1), (4, 3, 0), (4, 3, 1)],
    3: [(0, 4, 0), (0, 4, 1), (4, 2, 0), (4, 2, 1), (6, 1, 0), (6, 1, 1)],
}
# image-0 row chunks: [0,18) feeds row-groups 0-1 (critical-chunk DMA
# completion is ~latency-bound, nearly independent of size, so the
# first chunk carries enough rows that later chunks always arrive
# before their row-groups are reached -- a stalled PE gap resets the
# HAM warmup window and costs far more than the bytes).
X0_CHUNKS = [(0, 18), (18, 42), (42, 58)]
N_WARM = 28     # N=128 warmup matmuls (~114ns each cold)

TRACE = False
TRACE_KWARGS = {}
LAST_RESULTS = None
_NC_CACHE = None


def _build():
    nc = bacc.Bacc("TRN2", debug=False, target_bir_lowering=False,
                   num_devices=N_CORES, enable_partition_id=False,
                   monotonic_sem_count=0)
    x_d = nc.dram_tensor("x", [IMG_PER_CORE, CIN, HP, HP],
                         mybir.dt.float16, kind="ExternalInput")
    w_d = nc.dram_tensor("w", [CIN, 2, 9 * 128 + 1],
                         mybir.dt.float16, kind="ExternalInput")
    o_d = nc.dram_tensor("o", [IMG_PER_CORE, 2, 128, H * H],
                         mybir.dt.float16, kind="ExternalOutput")

    with tile.TileContext(nc) as tc:
        with (tc.tile_pool(name="wb", bufs=10) as wbp,
              tc.tile_pool(name="x", bufs=4) as xp,
              tc.tile_pool(name="o", bufs=8) as op,
              tc.tile_pool(name="ps", bufs=8, space="PSUM") as pp):
            wt = wbp.tile([CIN, 2, 9 * 128 + 1], mybir.dt.float16, tag="wtap")
            xts = [xp.tile([CIN, HP, HP], mybir.dt.float16,
                           tag="ximg", name=f"ximg{k}") for k in range(2)]
            gt = wbp.tile([CIN, 128], mybir.dt.float16, tag="warm")

            # Head DMAs, critical-first. x on the sync group, w on the
            # scalar group: triggers issue in parallel on the two
            # engines and each group's per-ring FIFO lands the first
            # chunk of each stream before the rest.
            for (a, b) in X0_CHUNKS:
                nc.sync.dma_start(xts[0][:, a:b, :], x_d.ap()[0, :, a:b, :])
            nc.sync.dma_start(xts[1][:], x_d.ap()[1])  # image-1 prefetch
            nc.scalar.dma_start(wt[:, 0, 0:384], w_d.ap()[:, 0, 0:384])
            nc.scalar.dma_start(wt[:, 0, 384:], w_d.ap()[:, 0, 384:])
            nc.scalar.dma_start(wt[:, 1], w_d.ap()[:, 1])

            # 8 PSUM accumulators (one bank each) rotating across blocks
            pss = [pp.tile([128, NFREE], mybir.dt.float32,
                           tag="psg", name=f"psg{k}") for k in range(8)]
            # output staging, one per (group, j) block in flight
            ots = [op.tile([128, 4 * NFREE], mybir.dt.float16,
                           tag="og", name=f"og{k}") for k in range(4)]

            # pre-warm the PE clock gate (HAM) with small matmuls on a
            # dedicated scratch tile while the first input DMAs are in
            # flight, so real matmuls run at 2.4 GHz almost from the
            # start. They write psum bank 0; the first real tap uses
            # start=True so the garbage never reaches an accumulation.
            # The memset rides gpsimd (the engine released earliest
            # from the preamble barrier) so warmups start ~6.5us with
            # no gap between the last warmup and the first real matmul
            # (a gap resets the HAM busy window).
            nc.gpsimd.memset(gt[:], 0.0)
            for _ in range(N_WARM):
                nc.tensor.matmul(pss[0][:, 0:128], gt[:], gt[:],
                                 start=True, stop=True)

            bank = 0
            blk = 0
            for img in range(IMG_PER_CORE):
                xt = xts[img % 2]
                for (g0, gsz, j) in BLOCKS[img]:
                    ot = ots[blk % 4]
                    bias_ap = wt[:, j, 9 * 128:9 * 128 + 1]
                    is_last = img == 3 and (g0, j) == (6, 1)
                    if is_last:
                        # final row group as two 4-row (N=224) PSUM
                        # sub-blocks: sub-A's whole act+DMA drain hides
                        # under sub-B's ~1.7us of matmuls (same total
                        # matmul cycles), leaving only the small sub-B
                        # flush exposed at the end.
                        hf = NFREE // 2
                        for h in range(2):
                            bnk = pss[(bank + h) % 8]
                            for t, (dy, dx) in enumerate(TAPS):
                                w_ap = wt[:, j, t * 128:(t + 1) * 128]
                                r0 = g0 * RG_ROWS + h * 4 + dy
                                nc.tensor.matmul(
                                    bnk[:, 0:hf], w_ap,
                                    xt[:, r0:r0 + 4, dx:dx + H],
                                    start=(t == 0), stop=(t == 8),
                                )
                            nc.scalar.activation(
                                ot[:, h * hf:(h + 1) * hf], bnk[:, 0:hf],
                                mybir.ActivationFunctionType.Relu,
                                bias=bias_ap)
                            nc.sync.dma_start(
                                o_d.ap()[img, j, :,
                                         g0 * NFREE + h * hf:
                                         g0 * NFREE + (h + 1) * hf],
                                ot[:, h * hf:(h + 1) * hf])
                        bank += 2
                        blk += 1
                        continue
                    banks = [pss[(bank + k) % 8] for k in range(gsz)]
                    bank += gsz
                    for t, (dy, dx) in enumerate(TAPS):
                        w_ap = wt[:, j, t * 128:(t + 1) * 128]
                        for k in range(gsz):
                            r0 = (g0 + k) * RG_ROWS + dy
                            nc.tensor.matmul(
                                banks[k][:], w_ap,
                                xt[:, r0:r0 + RG_ROWS, dx:dx + H],
                                start=(t == 0), stop=(t == 8),
                            )
                    for k in range(gsz):
                        nc.scalar.activation(
                            ot[:, k * NFREE:(k + 1) * NFREE], banks[k][:],
                            mybir.ActivationFunctionType.Relu,
                            bias=bias_ap)
                    # alternate output DMAs across the gpsimd and
                    # scalar rings; sync stays clear for input
                    # prefetches and the final drain
                    oeng = nc.gpsimd if blk % 2 == 0 else nc.scalar
                    blk += 1
                    oeng.dma_start(
                        o_d.ap()[img, j, :,
                                 g0 * NFREE:(g0 + gsz) * NFREE],
                        ot[:, 0:gsz * NFREE])
                # Prefetch image img+2 only after this image's blocks:
                # its WAR wait (previous reads of the same buffer) is
                # then already satisfied when it reaches the FIFO head,
                # so it never blocks the final drain DMAs in the sync
                # FIFO.
                if img < 2:
                    nc.sync.dma_start(xts[img % 2][:], x_d.ap()[img + 2])
    nc.compile()
    return nc


def kernel(prev_a, filter_w, filter_b):
    global LAST_RESULTS, _NC_CACHE
    from concourse.bass_utils import run_bass_kernel_spmd

    prev_a = np.asarray(prev_a, dtype=np.float32)
    filter_w = np.asarray(filter_w, dtype=np.float32)
    filter_b = np.asarray(filter_b, dtype=np.float32)

    n = prev_a.shape[0]
    xpad = np.zeros((n, CIN, HP, HP), dtype=np.float16)
    xpad[:, :, 1:1 + H, 1:1 + H] = prev_a.transpose(0, 3, 1, 2).astype(np.float16)
    # [cin, j-half, tap*128 + bias] so each cout half (taps + its bias
    # column) is one contiguous region
    w = np.zeros((CIN, 2, 9 * 128 + 1), dtype=np.float16)
    w[:, :, :9 * 128] = (filter_w.transpose(2, 0, 1, 3)
                         .reshape(CIN, 9, 2, 128).transpose(0, 2, 1, 3)
                         .reshape(CIN, 2, 9 * 128).astype(np.float16))
    w[:, :, 9 * 128] = filter_b.reshape(2, 128).T.astype(np.float16)

    if _NC_CACHE is None:
        _NC_CACHE = _build()
    nc = _NC_CACHE

    in_maps = [
        {"x": np.ascontiguousarray(xpad[c * IMG_PER_CORE:(c + 1) * IMG_PER_CORE]),
         "w": w}
        for c in range(N_CORES)
    ]
    LAST_RESULTS = run_bass_kernel_spmd(
        nc, in_maps, core_ids=list(range(N_CORES)), trace=TRACE,
        **TRACE_KWARGS)

    outs = []
    for c in range(N_CORES):
        o = LAST_RESULTS.results[c]["o"]  # [4, 2, 128, 3136] fp16
        outs.append(o.astype(np.float32)
                    .reshape(IMG_PER_CORE, COUT, H, H).transpose(0, 2, 3, 1))
    return np.ascontiguousarray(np.concatenate(outs, axis=0))
